# revision 38
# baseline (speedup 1.0000x reference)
"""HardMoE classifier forward on 8 Trainium2 NeuronCores (Bass/Tile).

Math (per row b of cls_token [B, D]):
    logits[j]  = cls_token[b] . Wcat[j],  j in 0..17
                 (Wcat = concat(gate_w [6,D], expert_w.reshape(12, D)))
    choice     = argmax(logits[0:6] + gate_b)      (first-index tiebreak)
    out[b, l]  = logits[6 + 2*choice + l] + expert_b[choice, l]

Strategy: pure data parallel over batch (8 cores x 16384 rows), fp32r PE.

This revision is DMA-bound at the measured platform floor (~291 GB/s/core
with all 8 cores streaming): 16 input DMAs x 14.4 us = ~231 us/pass, with
all compute hidden under the stream (DMA-only probe of the same access
pattern also measures ~231 us).

DMA layout: partition p owns the contiguous row block [128p, 128(p+1));
each input DMA loads [128, 8, 1024] (4 MiB, 32 KB contiguous/partition;
32 KB descriptors beat 16 KB by ~10% measured). Consecutive supers
alternate the sync/scalar HWDGE rings. A "sequential-block" mapping where
one DMA reads contiguous HBM measured ~11% SLOWER than this partition-
strided one. Consts ride the scalar ring so the first sync-ring
instruction is the first big input DMA. Outputs are staged in SBUF
[128, 128, 2] and written by ONE contiguous DMA per pass (row = 128p + m).

Compute per core, in super-iterations of 1024 rows (one input DMA each),
everything on the PE in float32r (the fast fp32 mode: 1 cycle/row matmul
vs 4 for fp32, 1.5 vs 2 for transpose; measured rel-err 1.35e-2 on this
input set from ~11/131072 argmax flips, within the 2e-2 gate):
  1. PE-transpose each [128,128] chunk; ACT/DVE copy psum->SBUF xT layout.
     (fp32r inputs must be "rounded-to-fp32r" by provenance: x/ident are
     declared float32r ExternalInputs; wt is rounded by an ACT copy.)
  2. fp32r matmuls accumulate logitsT [18, 512] per 512-row block at PSUM
     partition 0 (fp32r constrains dst partition; no tile_position).
  3. ACT drains logitsT to SBUF with the per-logit bias folded in
     (activation Identity + per-partition bias); PE-transposes strips back
     to [rows, 18]; vector engine does argmax with first-index tiebreak
     (desc weights), one-hot, 2-logit gather, straight from PSUM.
"""

import json

import numpy as np

import concourse.bass as bass
import concourse.mybir as mybir
from concourse.bass_utils import run_bass_kernel_spmd
from concourse.tile import TileContext

F32 = mybir.dt.float32
F32R = mybir.dt.float32r
ALU = mybir.AluOpType
AX = mybir.AxisListType

B, D, E, L = 131072, 1024, 6, 2
NCORES = 8
BLOC = B // NCORES            # 16384 rows per core
NJ = E + E * L                # 18 logit columns (6 gate + 12 expert)
KC = D // 128                 # 8 contraction chunks
NBLK = 2                      # 512-row matmul blocks per super
SUP = NBLK * 512              # 1024 rows per super-iteration = one input DMA
NSUP = BLOC // SUP            # 16 super-iterations per core
GT = 8                        # row-tiles per input DMA ([128, GT*1024] = 4 MiB)

# ---------------------------------------------------------------------------
# Workaround: this walrus build supports only ONE sync wait per instruction,
# but Tile emits instructions (and its tail drain) with several. Split the
# extra monotonic (sem-ge) waits onto single-wait NoOps placed immediately
# before the instruction on the same engine.
# ---------------------------------------------------------------------------
_wsplit_counter = [0]


def _split_multiwaits(mod: dict) -> dict:
    for fn in mod.get("functions", []):
        for blk in fn.get("blocks", []):
            out = []
            changed = False
            for ins in blk.get("instructions", []):
                si = ins.get("sync_info") or {}
                waits = si.get("on_wait") or []
                if len(waits) > 1:
                    changed = True
                    ge = [w for w in waits if w.get("wait_mode", "").startswith("sem-ge")]
                    rest = [w for w in waits if not w.get("wait_mode", "").startswith("sem-ge")]
                    assert len(rest) <= 1, (
                        f"multiple non-monotonic waits on {ins.get('name')}: {rest}"
                    )
                    keep = rest[0] if rest else ge.pop()
                    for w in ge:
                        _wsplit_counter[0] += 1
                        out.append({
                            "debug": ins.get("debug", 0),
                            "engine": ins["engine"],
                            "ins": [],
                            "name": f"WSPLIT-{_wsplit_counter[0]}",
                            "opcode": "NoOp",
                            "outs": [],
                            "sync_info": {"on_update": [], "on_wait": [w]},
                        })
                    si["on_wait"] = [keep]
                    ins["sync_info"] = si
                out.append(ins)
            if changed:
                blk["instructions"] = out
    return mod


_orig_to_json_bytes = bass.Bass.to_json_bytes


def _patched_to_json_bytes(self) -> bytes:
    mod = json.loads(_orig_to_json_bytes(self))
    return json.dumps(_split_multiwaits(mod)).encode()


if bass.Bass.to_json_bytes is not _patched_to_json_bytes:
    bass.Bass.to_json_bytes = _patched_to_json_bytes


# ---------------------------------------------------------------------------
# Device kernel (one NeuronCore's shard)
# ---------------------------------------------------------------------------

def _build_nc(time_loop: int = 0) -> bass.Bass:
    nc = bass.Bass(name="hardmoe")
    x = nc.dram_tensor("x", [BLOC, D], F32R, kind="ExternalInput")
    wt = nc.dram_tensor("wt", [KC, 128, NJ], F32, kind="ExternalInput")
    bias = nc.dram_tensor("bias", [32, 1], F32, kind="ExternalInput")
    desc = nc.dram_tensor("desc", [128, E], F32, kind="ExternalInput")
    idt = nc.dram_tensor("idt", [128, 128], F32R, kind="ExternalInput")
    idt32 = nc.dram_tensor("idt32", [128, NJ], F32, kind="ExternalInput")
    out = nc.dram_tensor("out", [BLOC, L], F32, kind="ExternalOutput")

    # contiguous view: partition p owns rows [128p, 128(p+1)); DMA n loads
    # rows r = 128p + GT*n + g as [128, GT, D], 32 KB contiguous/partition
    xv = x.rearrange("(p n g) d -> n p g d", p=128, g=GT)

    with TileContext(nc) as tc:
        with tc.tile_pool(name="const", bufs=1) as cpool, \
             tc.tile_pool(name="xin", bufs=3) as xpool, \
             tc.tile_pool(name="xt", bufs=2) as xtpool, \
             tc.tile_pool(name="pstr", bufs=4, space="PSUM") as pstr_pool, \
             tc.tile_pool(name="psmm", bufs=1, space="PSUM") as psmm_pool, \
             tc.tile_pool(name="pstb", bufs=2, space="PSUM") as pstb_pool, \
             tc.tile_pool(name="lsb", bufs=2) as lpool, \
             tc.tile_pool(name="sel", bufs=2) as selpool, \
             tc.tile_pool(name="ost", bufs=2) as opool:

            # consts ride the scalar (ACT) HWDGE ring so the sync ring's
            # first instruction is the first 4 MiB input DMA
            wt_f32 = cpool.tile([128, KC, NJ], F32)
            nc.scalar.dma_start(wt_f32[:], wt.rearrange("k p j -> p k j"))
            wt_sb = cpool.tile([128, KC, NJ], F32R)
            nc.scalar.copy(wt_sb[:], wt_f32[:])
            bias_sb = cpool.tile([32, 1], F32)
            nc.scalar.dma_start(bias_sb[:], bias[:])
            desc_sb = cpool.tile([128, E], F32)
            nc.scalar.dma_start(desc_sb[:], desc[:])
            ident = cpool.tile([128, 128], F32R)
            nc.scalar.dma_start(ident[:], idt[:])
            ident32 = cpool.tile([128, NJ], F32)
            nc.scalar.dma_start(ident32[:], idt32[:])

            def body():
                ost = opool.tile([128, BLOC // 128, L], F32, tag="ost")

                def stage_super(s: int):
                    """One super = 1024 rows = one 4 MiB DMA of GT=8 tiles."""
                    xts = xtpool.tile([128, KC, SUP], F32R, tag="xts")
                    xb = xpool.tile([128, GT, D], F32R, tag="xb")
                    eng = nc.sync if s % 2 == 0 else nc.scalar
                    eng.dma_start(xb[:], xv[s])
                    for g in range(GT):
                        for h in range(2):
                            pst = pstr_pool.tile([128, 512], F32R, tag="pst")
                            for q in range(4):
                                k = h * 4 + q
                                nc.tensor.transpose(
                                    pst[:, q * 128:(q + 1) * 128],
                                    xb[:, g, k * 128:(k + 1) * 128],
                                    ident[:],
                                )
                            dst = xts[:, h * 4:(h + 1) * 4,
                                      g * 128:(g + 1) * 128]
                            if (g + h) % 2 == 0:
                                nc.scalar.copy(dst, pst[:])
                            else:
                                nc.vector.tensor_copy(dst, pst[:])
                    return xts

                live = {0: stage_super(0), 1: stage_super(1)}

                for s in range(NSUP):
                    xts = live.pop(s)

                    # fp32r matmuls: block j <-> xts columns [512j, 512(j+1));
                    # each block accumulates logitsT [18, 512] at partition 0
                    # of its own PSUM bank (fp32r needs dst partition 0)
                    ps_mm = [
                        psmm_pool.tile([32, 512], F32, tag=f"ps_mm{j}",
                                       name=f"ps_mm{j}")
                        for j in range(NBLK)
                    ]
                    for k in range(KC):
                        for j in range(NBLK):
                            nc.tensor.matmul(
                                ps_mm[j][0:NJ, :],
                                wt_sb[:, k],
                                xts[:, k, 512 * j:512 * (j + 1)],
                                start=(k == 0),
                                stop=(k == KC - 1),
                            )
                    if s + 2 < NSUP:
                        live[s + 2] = stage_super(s + 2)
                    # drain PSUM -> SBUF with the per-logit bias folded in
                    l_sb = lpool.tile([32, NBLK, 512], F32, tag="l_sb")
                    for j in range(NBLK):
                        nc.scalar.activation(
                            l_sb[0:NJ, j, :],
                            ps_mm[j][0:NJ, :],
                            mybir.ActivationFunctionType.Identity,
                            bias=bias_sb[0:NJ],
                        )

                    # transpose logitsT strips back to [rows, 18] and select
                    tp = pstb_pool.tile([128, 8, NJ], F32, tag="tp")
                    for half in range(8):                 # 8 x 128-row slices
                        j = half // 4
                        c = half % 4
                        nc.tensor.matmul(
                            tp[:, half, :],
                            l_sb[0:NJ, j, c * 128:(c + 1) * 128],
                            ident32[0:NJ, :],
                            is_transpose=True,
                        )
                    gate = tp[:, :, 0:E]
                    m = selpool.tile([128, 8], F32, tag="m")
                    nc.vector.tensor_reduce(m[:], gate, AX.X, ALU.max)
                    eq = selpool.tile([128, 8, E], F32, tag="eq")
                    nc.vector.tensor_tensor(
                        eq[:], gate, m[:, :, None].to_broadcast([128, 8, E]),
                        ALU.is_ge,
                    )
                    nc.vector.tensor_tensor(
                        eq[:], eq[:],
                        desc_sb[:, None, :].to_broadcast([128, 8, E]),
                        ALU.mult,
                    )
                    nc.vector.tensor_reduce(m[:], eq[:], AX.X, ALU.max)
                    onehot = selpool.tile([128, 8, E], F32, tag="onehot")
                    nc.vector.tensor_tensor(
                        onehot[:], eq[:],
                        m[:, :, None].to_broadcast([128, 8, E]),
                        ALU.is_equal,
                    )
                    sel = selpool.tile([128, 8, E], F32, tag="sel")
                    # rows r = 128p + 8s + half -> ost[p, 8s + half, l]
                    c0 = s * (SUP // 128)
                    for l in range(L):
                        nc.vector.tensor_tensor(
                            sel[:], onehot[:], tp[:, :, E + l::L], ALU.mult
                        )
                        nc.vector.tensor_reduce(
                            ost[:, c0:c0 + 8, l], sel[:], AX.X, ALU.add
                        )
                # one contiguous output DMA per pass (row = 128p + m)
                nc.sync.dma_start(
                    out.rearrange("(p m) l -> p m l", p=128), ost[:])

            if time_loop:
                with tc.For_i(0, time_loop, 1, name="timing") as _i:
                    body()
            else:
                body()
    return nc


_cached = None


def _get_nc() -> bass.Bass:
    global _cached
    if _cached is None:
        _cached = _build_nc()
    return _cached


# ---------------------------------------------------------------------------
# Host wrapper
# ---------------------------------------------------------------------------

def _host_inputs(cls_token, gate_w, gate_b, expert_w, expert_b):
    x = np.ascontiguousarray(np.asarray(cls_token, dtype=np.float32))
    gw = np.asarray(gate_w, dtype=np.float32)
    gb = np.asarray(gate_b, dtype=np.float32)
    ew = np.asarray(expert_w, dtype=np.float32)
    eb = np.asarray(expert_b, dtype=np.float32)
    assert x.shape == (B, D), x.shape

    wcat = np.concatenate([gw, ew.reshape(E * L, D)], axis=0)      # [18, D]
    wt_in = np.ascontiguousarray(wcat.T).reshape(KC, 128, NJ)
    bias_in = np.zeros((32, 1), np.float32)
    bias_in[:NJ, 0] = np.concatenate([gb, eb.reshape(E * L)])
    desc_in = np.ascontiguousarray(np.broadcast_to(
        (E - np.arange(E, dtype=np.float32))[None, :], (128, E)))
    idt_in = np.eye(128, dtype=np.float32)
    idt32_in = np.zeros((128, NJ), np.float32)
    for p in range(NJ):
        idt32_in[p, p] = 1.0

    in_maps = []
    for c in range(NCORES):
        in_maps.append({
            "x": x[c * BLOC:(c + 1) * BLOC],
            "wt": wt_in,
            "bias": bias_in,
            "desc": desc_in,
            "idt": idt_in,
            "idt32": idt32_in,
        })
    return in_maps


def kernel(cls_token, gate_w, gate_b, expert_w, expert_b) -> np.ndarray:
    in_maps = _host_inputs(cls_token, gate_w, gate_b, expert_w, expert_b)
    res = run_bass_kernel_spmd(_get_nc(), in_maps, core_ids=list(range(NCORES)))
    return np.concatenate([r["out"] for r in res.results], axis=0)



# revision 39
# speedup vs baseline: 1.0151x; 1.0151x over previous
"""HardMoE classifier forward on 8 Trainium2 NeuronCores (Bass/Tile).

Math (per row b of cls_token [B, D]):
    logits[j]  = cls_token[b] . Wcat[j],  j in 0..17
                 (Wcat = concat(gate_w [6,D], expert_w.reshape(12, D)))
    choice     = argmax(logits[0:6] + gate_b)      (first-index tiebreak)
    out[b, l]  = logits[6 + 2*choice + l] + expert_b[choice, l]

Strategy: pure data parallel over batch (8 cores x 16384 rows), fp32r PE.

This revision is DMA-bound at the measured platform floor (~291 GB/s/core
with all 8 cores streaming): 16 input DMAs x 14.4 us = ~231 us/pass, with
all compute hidden under the stream (DMA-only probe of the same access
pattern also measures ~231 us).

DMA layout: partition p owns the contiguous row block [128p, 128(p+1));
each input DMA loads [128, 8, 1024] (4 MiB, 32 KB contiguous/partition;
32 KB descriptors beat 16 KB by ~10% measured). Consecutive supers
alternate the sync/scalar HWDGE rings. A "sequential-block" mapping where
one DMA reads contiguous HBM measured ~11% SLOWER than this partition-
strided one. Consts ride the scalar ring so the first sync-ring
instruction is the first big input DMA. Outputs are staged in SBUF
[128, 128, 2] and written by ONE contiguous DMA per pass (row = 128p + m).

Compute per core, in super-iterations of 1024 rows (one input DMA each),
everything on the PE in float32r (the fast fp32 mode: 1 cycle/row matmul
vs 4 for fp32, 1.5 vs 2 for transpose; measured rel-err 1.35e-2 on this
input set from ~11/131072 argmax flips, within the 2e-2 gate):
  1. PE-transpose each [128,128] chunk; ACT/DVE copy psum->SBUF xT layout.
     (fp32r inputs must be "rounded-to-fp32r" by provenance: x/ident are
     declared float32r ExternalInputs; wt is rounded by an ACT copy.)
  2. fp32r matmuls accumulate logitsT [18, 512] per 512-row block at PSUM
     partition 0 (fp32r constrains dst partition; no tile_position).
  3. ACT drains logitsT to SBUF with the per-logit bias folded in
     (activation Identity + per-partition bias); PE-transposes strips back
     to [rows, 18]; vector engine does argmax with first-index tiebreak
     (desc weights), one-hot, 2-logit gather, straight from PSUM.
"""

import json

import numpy as np

import concourse.bass as bass
import concourse.mybir as mybir
from concourse.bass_utils import run_bass_kernel_spmd
from concourse.tile import TileContext

F32 = mybir.dt.float32
F32R = mybir.dt.float32r
ALU = mybir.AluOpType
AX = mybir.AxisListType

B, D, E, L = 131072, 1024, 6, 2
NCORES = 8
BLOC = B // NCORES            # 16384 rows per core
NJ = E + E * L                # 18 logit columns (6 gate + 12 expert)
KC = D // 128                 # 8 contraction chunks
NBLK = 2                      # 512-row matmul blocks per super
SUP = NBLK * 512              # 1024 rows per super-iteration = one input DMA
NSUP = BLOC // SUP            # 16 super-iterations per core
GT = 8                        # row-tiles per input DMA ([128, GT*1024] = 4 MiB)

# ---------------------------------------------------------------------------
# Workaround: this walrus build supports only ONE sync wait per instruction,
# but Tile emits instructions (and its tail drain) with several. Split the
# extra monotonic (sem-ge) waits onto single-wait NoOps placed immediately
# before the instruction on the same engine.
# ---------------------------------------------------------------------------
_wsplit_counter = [0]


def _split_multiwaits(mod: dict) -> dict:
    for fn in mod.get("functions", []):
        for blk in fn.get("blocks", []):
            out = []
            changed = False
            for ins in blk.get("instructions", []):
                si = ins.get("sync_info") or {}
                waits = si.get("on_wait") or []
                if len(waits) > 1:
                    changed = True
                    ge = [w for w in waits if w.get("wait_mode", "").startswith("sem-ge")]
                    rest = [w for w in waits if not w.get("wait_mode", "").startswith("sem-ge")]
                    assert len(rest) <= 1, (
                        f"multiple non-monotonic waits on {ins.get('name')}: {rest}"
                    )
                    keep = rest[0] if rest else ge.pop()
                    for w in ge:
                        _wsplit_counter[0] += 1
                        out.append({
                            "debug": ins.get("debug", 0),
                            "engine": ins["engine"],
                            "ins": [],
                            "name": f"WSPLIT-{_wsplit_counter[0]}",
                            "opcode": "NoOp",
                            "outs": [],
                            "sync_info": {"on_update": [], "on_wait": [w]},
                        })
                    si["on_wait"] = [keep]
                    ins["sync_info"] = si
                out.append(ins)
            if changed:
                blk["instructions"] = out
    return mod


_orig_to_json_bytes = bass.Bass.to_json_bytes


def _patched_to_json_bytes(self) -> bytes:
    mod = json.loads(_orig_to_json_bytes(self))
    return json.dumps(_split_multiwaits(mod)).encode()


if bass.Bass.to_json_bytes is not _patched_to_json_bytes:
    bass.Bass.to_json_bytes = _patched_to_json_bytes


# ---------------------------------------------------------------------------
# Device kernel (one NeuronCore's shard)
# ---------------------------------------------------------------------------

def _build_nc(time_loop: int = 0) -> bass.Bass:
    nc = bass.Bass(name="hardmoe")
    x = nc.dram_tensor("x", [BLOC, D], F32R, kind="ExternalInput")
    wt = nc.dram_tensor("wt", [KC, 128, NJ], F32, kind="ExternalInput")
    bias = nc.dram_tensor("bias", [32, 1], F32, kind="ExternalInput")
    desc = nc.dram_tensor("desc", [128, E], F32, kind="ExternalInput")
    idt = nc.dram_tensor("idt", [128, 128], F32R, kind="ExternalInput")
    idt32 = nc.dram_tensor("idt32", [128, NJ], F32, kind="ExternalInput")
    out = nc.dram_tensor("out", [BLOC, L], F32, kind="ExternalOutput")

    # contiguous view: partition p owns rows [128p, 128(p+1)); DMA n loads
    # rows r = 128p + GT*n + g as [128, GT, D], 32 KB contiguous/partition
    xv = x.rearrange("(p n g) d -> n p g d", p=128, g=GT)

    with TileContext(nc) as tc:
        with tc.tile_pool(name="const", bufs=1) as cpool, \
             tc.tile_pool(name="xin", bufs=3) as xpool, \
             tc.tile_pool(name="xt", bufs=2) as xtpool, \
             tc.tile_pool(name="pstr", bufs=4, space="PSUM") as pstr_pool, \
             tc.tile_pool(name="psmm", bufs=1, space="PSUM") as psmm_pool, \
             tc.tile_pool(name="pstb", bufs=2, space="PSUM") as pstb_pool, \
             tc.tile_pool(name="lsb", bufs=2) as lpool, \
             tc.tile_pool(name="sel", bufs=2) as selpool, \
             tc.tile_pool(name="ost", bufs=2) as opool:

            # consts ride the scalar (ACT) HWDGE ring so the sync ring's
            # first instruction is the first 4 MiB input DMA
            wt_f32 = cpool.tile([128, KC, NJ], F32)
            nc.scalar.dma_start(wt_f32[:], wt.rearrange("k p j -> p k j"))
            wt_sb = cpool.tile([128, KC, NJ], F32R)
            nc.scalar.copy(wt_sb[:], wt_f32[:])
            bias_sb = cpool.tile([32, 1], F32)
            nc.scalar.dma_start(bias_sb[:], bias[:])
            desc_sb = cpool.tile([128, E], F32)
            nc.scalar.dma_start(desc_sb[:], desc[:])
            ident = cpool.tile([128, 128], F32R)
            nc.scalar.dma_start(ident[:], idt[:])
            ident32 = cpool.tile([128, NJ], F32)
            nc.scalar.dma_start(ident32[:], idt32[:])

            def body():
                ost = opool.tile([128, BLOC // 128, L], F32, tag="ost")

                def stage_super(s: int):
                    """One super = 1024 rows = one 4 MiB DMA of GT=8 tiles."""
                    xts = xtpool.tile([128, KC, SUP], F32R, tag="xts")
                    xb = xpool.tile([128, GT, D], F32R, tag="xb")
                    eng = nc.sync if s % 2 == 0 else nc.scalar
                    eng.dma_start(xb[:], xv[s])
                    for g in range(GT):
                        for h in range(2):
                            pst = pstr_pool.tile([128, 512], F32R, tag="pst")
                            for q in range(4):
                                k = h * 4 + q
                                nc.tensor.transpose(
                                    pst[:, q * 128:(q + 1) * 128],
                                    xb[:, g, k * 128:(k + 1) * 128],
                                    ident[:],
                                )
                            dst = xts[:, h * 4:(h + 1) * 4,
                                      g * 128:(g + 1) * 128]
                            if (g + h) % 2 == 0:
                                nc.scalar.copy(dst, pst[:])
                            else:
                                nc.vector.tensor_copy(dst, pst[:])
                    return xts

                live = {0: stage_super(0), 1: stage_super(1)}

                for s in range(NSUP):
                    xts = live.pop(s)

                    # fp32r matmuls: block j <-> xts columns [512j, 512(j+1));
                    # each block accumulates logitsT [18, 512] at partition 0
                    # of its own PSUM bank (fp32r needs dst partition 0)
                    ps_mm = [
                        psmm_pool.tile([32, 512], F32, tag=f"ps_mm{j}",
                                       name=f"ps_mm{j}")
                        for j in range(NBLK)
                    ]
                    for k in range(KC):
                        for j in range(NBLK):
                            nc.tensor.matmul(
                                ps_mm[j][0:NJ, :],
                                wt_sb[:, k],
                                xts[:, k, 512 * j:512 * (j + 1)],
                                start=(k == 0),
                                stop=(k == KC - 1),
                            )
                    if s + 2 < NSUP:
                        live[s + 2] = stage_super(s + 2)
                    # drain PSUM -> SBUF with the per-logit bias folded in
                    l_sb = lpool.tile([32, NBLK, 512], F32, tag="l_sb")
                    for j in range(NBLK):
                        nc.scalar.activation(
                            l_sb[0:NJ, j, :],
                            ps_mm[j][0:NJ, :],
                            mybir.ActivationFunctionType.Identity,
                            bias=bias_sb[0:NJ],
                        )

                    # transpose logitsT strips back to [rows, 18] and select
                    tp = pstb_pool.tile([128, 8, NJ], F32, tag="tp")
                    for half in range(8):                 # 8 x 128-row slices
                        j = half // 4
                        c = half % 4
                        nc.tensor.matmul(
                            tp[:, half, :],
                            l_sb[0:NJ, j, c * 128:(c + 1) * 128],
                            ident32[0:NJ, :],
                            is_transpose=True,
                        )
                    gate = tp[:, :, 0:E]
                    m = selpool.tile([128, 8], F32, tag="m")
                    nc.vector.tensor_reduce(m[:], gate, AX.X, ALU.max)
                    eq = selpool.tile([128, 8, E], F32, tag="eq")
                    nc.vector.tensor_tensor(
                        eq[:], gate, m[:, :, None].to_broadcast([128, 8, E]),
                        ALU.is_ge,
                    )
                    nc.vector.tensor_tensor(
                        eq[:], eq[:],
                        desc_sb[:, None, :].to_broadcast([128, 8, E]),
                        ALU.mult,
                    )
                    nc.vector.tensor_reduce(m[:], eq[:], AX.X, ALU.max)
                    onehot = selpool.tile([128, 8, E], F32, tag="onehot")
                    nc.vector.tensor_tensor(
                        onehot[:], eq[:],
                        m[:, :, None].to_broadcast([128, 8, E]),
                        ALU.is_equal,
                    )
                    sel = selpool.tile([128, 8, E], F32, tag="sel")
                    # rows r = 128p + 8s + half -> ost[p, 8s + half, l]
                    c0 = s * (SUP // 128)
                    for l in range(L):
                        nc.vector.tensor_tensor(
                            sel[:], onehot[:], tp[:, :, E + l::L], ALU.mult
                        )
                        nc.vector.tensor_reduce(
                            ost[:, c0:c0 + 8, l], sel[:], AX.X, ALU.add
                        )
                # one contiguous output DMA per pass (row = 128p + m), on the
                # scalar ring: the sync ring's FIFO then never delays the
                # next iteration's first input DMA behind the output's
                # ~2 us completion
                nc.scalar.dma_start(
                    out.rearrange("(p m) l -> p m l", p=128), ost[:])

            if time_loop:
                with tc.For_i(0, time_loop, 1, name="timing") as _i:
                    body()
            else:
                body()
    return nc


_cached = None


def _get_nc() -> bass.Bass:
    global _cached
    if _cached is None:
        _cached = _build_nc()
    return _cached


# ---------------------------------------------------------------------------
# Host wrapper
# ---------------------------------------------------------------------------

def _host_inputs(cls_token, gate_w, gate_b, expert_w, expert_b):
    x = np.ascontiguousarray(np.asarray(cls_token, dtype=np.float32))
    gw = np.asarray(gate_w, dtype=np.float32)
    gb = np.asarray(gate_b, dtype=np.float32)
    ew = np.asarray(expert_w, dtype=np.float32)
    eb = np.asarray(expert_b, dtype=np.float32)
    assert x.shape == (B, D), x.shape

    wcat = np.concatenate([gw, ew.reshape(E * L, D)], axis=0)      # [18, D]
    wt_in = np.ascontiguousarray(wcat.T).reshape(KC, 128, NJ)
    bias_in = np.zeros((32, 1), np.float32)
    bias_in[:NJ, 0] = np.concatenate([gb, eb.reshape(E * L)])
    desc_in = np.ascontiguousarray(np.broadcast_to(
        (E - np.arange(E, dtype=np.float32))[None, :], (128, E)))
    idt_in = np.eye(128, dtype=np.float32)
    idt32_in = np.zeros((128, NJ), np.float32)
    for p in range(NJ):
        idt32_in[p, p] = 1.0

    in_maps = []
    for c in range(NCORES):
        in_maps.append({
            "x": x[c * BLOC:(c + 1) * BLOC],
            "wt": wt_in,
            "bias": bias_in,
            "desc": desc_in,
            "idt": idt_in,
            "idt32": idt32_in,
        })
    return in_maps


def kernel(cls_token, gate_w, gate_b, expert_w, expert_b) -> np.ndarray:
    in_maps = _host_inputs(cls_token, gate_w, gate_b, expert_w, expert_b)
    res = run_bass_kernel_spmd(_get_nc(), in_maps, core_ids=list(range(NCORES)))
    return np.concatenate([r["out"] for r in res.results], axis=0)

